# revision 3
# baseline (speedup 1.0000x reference)
"""Chamfer distance loss kernel for Trainium2 (8 NeuronCores, data-parallel over batch).

Strategy (v2 — fp8 DoubleRow matmul + 3-engine min pipeline):
  - B=16 batches sharded 2 per core across 8 cores.
  - Per batch, d2[n, m] = ||p_n||^2 + ||g_m||^2 - 2 p_n . g_m via an augmented
    matmul. Operands are fp8(e4m3) with a 3-term hi/lo/lolo split per value and
    6 cross-term pairs for the coordinate rows; the squared-norm rows are
    scaled by lam=8 against exact 1/lam partner rows to dodge the fp8 denormal
    floor, and the -2*p.g cross term is balanced as (sqrt2*p).(-sqrt2*g).
    K = 36 rows packed as 2 k-tiles of 18 -> a single fp8 DoubleRow matmul per
    512-col m-block (0.5 PE cycles/col: half the fp16 cost, and fast enough
    that the PE p-state ramp never becomes the critical path).
    Measured end-to-end loss rel-err of this quantization: ~6.6e-3 (tol 2e-2).
  - All reductions run NEGATED (cp = -d2) so that every min becomes a max;
    this lets the GPSIMD/Pool engine participate via partition_all_reduce
    (its only reduce ops are add/max/absmax, and it cannot touch PSUM).
  - Each [128, 2048] fp32 PSUM chunk is drained exactly once, by one of:
      * ACT negate-copy -> fp16 SBUF cp, then a DVE tensor_scalar accum-max
        (4x mode) produces the z2 column (A-chunks), or
      * DVE tensor_scalar (op0=mult -1) directly from PSUM (1x) producing BOTH
        the fp16 cp and the z2 column in one pass (V-chunks).
  - z path (min over n): split across two engines per batch:
      * S_D chunks: DVE elementwise max chain (2x mode) into acc_d,
      * S_P chunks: independent Pool partition_all_reduce(max) per chunk; row
        0 of each result is DMA'd out ([1, 2048] partial).
    The host merges: z = -max(acc_d partition-max, all S_P partials).
  - Host takes sqrt of the min-d2 values and sums into the scalar loss.
  This keeps ACT (~1.9us/chunk), DVE (~2.0us/chunk avg) and Pool (~2.9us
  /chunk on its 8) all busy in parallel instead of serializing the drain on
  ACT as the previous version did.
"""

import math

import numpy as np
import ml_dtypes

import concourse.bass as bass
import concourse.tile as tile
from concourse import bacc, bass_utils, mybir
from concourse.bass_isa import ReduceOp

B = 16  # total batches
NCORES = 8
BPC = B // NCORES  # batches per core
N = 2048  # points per cloud
NCHUNK = 16  # chunks of 128 predict points
MBLK = 4  # m-blocks of 512 gt points per chunk
KT = 18  # K rows per k-tile (2 tiles -> K=36)

F32 = mybir.dt.float32
FP16 = mybir.dt.float16
FP8 = mybir.dt.float8e4
E4M3 = ml_dtypes.float8_e4m3  # numpy dtype matching mybir float8e4
MIN = mybir.AluOpType.min
MAX = mybir.AluOpType.max
MULT = mybir.AluOpType.mult
DR = mybir.MatmulPerfMode.DoubleRow
NEG_BIG = -60000.0  # max-identity (all -d2 values are >> this)

LAM = 8.0  # scale for the squared-norm rows (max lam*p^2 ~ 160 < 240 fp8 max)
SQ2 = math.sqrt(2.0)

# per-batch chunk classification (see module docstring)
V_CHUNKS = (3, 7, 11, 15)  # DVE-fused drains; the rest are ACT drains
POOL_CHUNKS = (1, 2, 4, 5, 6, 8, 9, 10)  # z-path via Pool partition_all_reduce
# remaining chunks {0, 3, 7, 11, 12, 13, 14, 15} feed the DVE max chain
NPOOL = len(POOL_CHUNKS)
LAST_DVE_CHAIN = 15


def _build_program():
    nc = bacc.Bacc("TRN2", target_bir_lowering=False, debug=False)
    # fp8 augmented operands: [r, t, c] = k-row r of k-tile t; columns 0:N are
    # the p-side (lhsT source), N:2N the g-side (rhs source)
    pg_in = nc.dram_tensor("pg_in", (BPC, KT, 2, 2 * N), FP8, kind="ExternalInput")
    # negated z2 maxes (per-predict-point -min d2): [b, p, i] = point i*128+p
    mins = nc.dram_tensor("mins", (BPC, 128, NCHUNK), F32, kind="ExternalOutput")
    # z-path partials (negated): DVE chain accumulator + Pool per-chunk rows
    accd = nc.dram_tensor("accd", (BPC, 128, N), FP16, kind="ExternalOutput")
    zp = nc.dram_tensor("zp", (BPC, NPOOL, N), FP16, kind="ExternalOutput")

    with tile.TileContext(nc) as tc:
        with (
            tc.tile_pool(name="aug", bufs=2) as aug_pool,
            tc.tile_pool(name="d2p", bufs=2, space="PSUM") as psum_pool,
            tc.tile_pool(name="cpp", bufs=8) as cp_pool,
            tc.tile_pool(name="junkp", bufs=3) as junk_pool,
            tc.tile_pool(name="accdp", bufs=2) as accd_pool,
            tc.tile_pool(name="parp", bufs=2) as par_pool,
            tc.tile_pool(name="outp", bufs=2) as out_pool,
        ):
            for b in range(BPC):
                # operand replicas at partition bases 0/32/64/96 so the four
                # m-block matmuls of a chunk run on distinct PE row groups
                aug = aug_pool.tile([128, 2, 2 * N], FP8, tag="aug")
                for g in range(MBLK):
                    nc.sync.dma_start(aug[32 * g : 32 * g + KT], pg_in[b])

                z2t = out_pool.tile([128, NCHUNK], F32, tag="z2")
                acc_d = accd_pool.tile([128, N], FP16, tag="accd")
                npar = 0

                for i in range(NCHUNK):
                    d2 = psum_pool.tile([128, N], F32, tag="d2")
                    for j in range(MBLK):
                        base = 32 * j
                        nc.tensor.matmul(
                            d2[:, j * 512 : (j + 1) * 512],
                            aug[base : base + KT, :, i * 128 : (i + 1) * 128],
                            aug[base : base + KT, :, N + j * 512 : N + (j + 1) * 512],
                            start=True,
                            stop=True,
                            perf_mode=DR,
                            tile_position=(base, 0),
                        )
                    # drain (negating) + z2 column
                    cp = cp_pool.tile([128, N], FP16, tag="cp")
                    if i in V_CHUNKS:
                        # one DVE pass: cp = -d2 (fp16) + accum-max z2 column
                        nc.vector.tensor_scalar(
                            cp[:], d2[:], -1.0, None,
                            op0=MULT, op1=MAX, accum_out=z2t[:, i : i + 1],
                        )
                    else:
                        nc.scalar.mul(cp[:], d2[:], -1.0)  # ACT negate-copy
                        if i == 0:
                            # elementwise out doubles as the DVE-chain init
                            nc.vector.tensor_scalar(
                                acc_d[:], cp[:], NEG_BIG, None,
                                op0=MAX, op1=MAX, accum_out=z2t[:, i : i + 1],
                            )
                        else:
                            junk = junk_pool.tile([128, N], FP16, tag="junk")
                            nc.vector.tensor_scalar(
                                junk[:], cp[:], NEG_BIG, None,
                                op0=MAX, op1=MAX, accum_out=z2t[:, i : i + 1],
                            )
                    # z path
                    if i in POOL_CHUNKS:
                        par = par_pool.tile([128, N], FP16, tag="par")
                        nc.gpsimd.partition_all_reduce(par[:], cp[:], 128, ReduceOp.max)
                        nc.sync.dma_start(zp[b][npar : npar + 1, :], par[0:1, :])
                        npar += 1
                    elif i != 0:  # DVE chain (chunk 0 initialized acc_d)
                        if i == LAST_DVE_CHAIN:
                            # split the last chain step so the accumulator DMA
                            # overlaps the second half's max
                            h = N // 2
                            nc.vector.tensor_tensor(
                                acc_d[:, 0:h], cp[:, 0:h], acc_d[:, 0:h], op=MAX
                            )
                            nc.sync.dma_start(accd[b][:, 0:h], acc_d[:, 0:h])
                            nc.vector.tensor_tensor(
                                acc_d[:, h:N], cp[:, h:N], acc_d[:, h:N], op=MAX
                            )
                            nc.sync.dma_start(accd[b][:, h:N], acc_d[:, h:N])
                        else:
                            nc.vector.tensor_tensor(acc_d[:], cp[:], acc_d[:], op=MAX)

                nc.sync.dma_start(mins[b], z2t[:])
    nc.compile()
    return nc


_NC_CACHE = None


def _get_nc():
    global _NC_CACHE
    if _NC_CACHE is None:
        _NC_CACHE = _build_program()
    return _NC_CACHE


def _split3(x):
    """3-term fp8(e4m3) split: x ~= t0 + t1 + t2 (fp8 arrays returned)."""
    terms = []
    r = x.astype(np.float32)
    for _ in range(3):
        q = r.astype(E4M3)
        terms.append(q)
        r = r - q.astype(np.float32)
    return terms


def _augment(predict_pc, gt_pc):
    """Host-side marshaling into the fp8 DoubleRow operand [B, KT, 2, 2N].

    Row blocks (3 rows each, one per coordinate), k-tile 0 then k-tile 1:
      tile 0: (sqp0|inv) (sqp1|inv) (sqp2|inv) (inv|sqg0) (inv|sqg1) (inv|sqg2)
      tile 1: (A0|B0) (A0|B1) (A1|B0) (A1|B1) (A0|B2) (A2|B0)
    with sqp = split3(lam*p^2), sqg = split3(lam*g^2), inv = 1/lam (exact),
    A = split3(sqrt2*p), B = split3(-sqrt2*g).
    """
    Bn = predict_pc.shape[0]
    p = predict_pc.astype(np.float32)
    g = gt_pc.astype(np.float32)
    sqp = _split3(LAM * p * p)
    sqg = _split3(LAM * g * g)
    A = _split3(np.float32(SQ2) * p)
    Bt = _split3(np.float32(-2.0 / SQ2) * g)
    inv = np.full_like(p, 1.0 / LAM).astype(E4M3)

    out = np.zeros((Bn, KT, 2, 2 * N), dtype=E4M3)
    for blk in range(6):  # k-tile 0: squared-norm rows
        if blk < 3:
            pa, gb = sqp[blk], inv
        else:
            pa, gb = inv, sqg[blk - 3]
        out[:, 3 * blk : 3 * blk + 3, 0, 0:N] = pa
        out[:, 3 * blk : 3 * blk + 3, 0, N : 2 * N] = gb
    pairs = [(0, 0), (0, 1), (1, 0), (1, 1), (0, 2), (2, 0)]
    for blk, (ia, jb) in enumerate(pairs):  # k-tile 1: coordinate rows
        out[:, 3 * blk : 3 * blk + 3, 1, 0:N] = A[ia]
        out[:, 3 * blk : 3 * blk + 3, 1, N : 2 * N] = Bt[jb]
    return np.ascontiguousarray(out)


def kernel(predict_pc, gt_pc):
    predict_pc = np.ascontiguousarray(np.asarray(predict_pc, dtype=np.float32))
    gt_pc = np.ascontiguousarray(np.asarray(gt_pc, dtype=np.float32))
    pg = _augment(predict_pc, gt_pc)
    nc = _get_nc()
    in_maps = [
        {"pg_in": np.ascontiguousarray(pg[c * BPC : (c + 1) * BPC])}
        for c in range(NCORES)
    ]
    res = bass_utils.run_bass_kernel_spmd(nc, in_maps, core_ids=list(range(NCORES)))
    total = 0.0
    for c in range(NCORES):
        m = np.asarray(res.results[c]["mins"], dtype=np.float64)  # [BPC, 128, 16]
        total += np.sqrt(np.maximum(-m, 0.0)).sum()
        ad = np.asarray(res.results[c]["accd"], dtype=np.float32)  # [BPC, 128, N]
        zpr = np.asarray(res.results[c]["zp"], dtype=np.float32)  # [BPC, NPOOL, N]
        zneg = np.maximum(ad.max(axis=1), zpr.max(axis=1))  # [BPC, N]
        total += np.sqrt(np.maximum(-zneg, 0.0), dtype=np.float64).sum()
    return np.float32(total / (B * N))
